# revision 12
# baseline (speedup 1.0000x reference)
"""Trainium2 Bass kernel for nn_Bilinear: out = (X @ W) @ X^T + b.

X: (8192, 1024) f32, W: (1024, 1024) f32, b: (1,) f32 -> out: (8192, 8192) f32.

Sharding: row-block over 8 cores. Core c computes rows [c*1024, (c+1)*1024).
Host passes X already transposed (XT, [d, N]) so the kernel needs no on-device
transposes:
  stage 1: XWT = W^T @ XlocT        (= (X_loc @ W)^T, [d', i])   via matmul(lhsT=W, rhs=XlocT)
  stage 2: out = XWT^T @ XT + b     ([i, j])                      via matmul(lhsT=XWT, rhs=XT)
Matmuls run as float32r (full-rate fp32 path on the PE for free-dim >= 256).
"""

import numpy as np

N = 8192
D = 1024
NCORES = 8
NLOC = N // NCORES  # 1024 rows per core
P = 128
KC = D // P  # 8 contraction chunks
FD = 512  # matmul free dim (one fp32 PSUM bank)
NJ = N // FD  # 16 output column slices
NI = NLOC // P  # 8 output row chunks per core

_CACHE = {}


def _build_nc(mm_dtype_name="float32r"):
    import concourse.mybir as mybir
    import concourse.tile as tile
    from concourse import bacc

    f32 = mybir.dt.float32
    mmdt = getattr(mybir.dt, mm_dtype_name)

    nc = bacc.Bacc(trn_type="TRN2")
    xt = nc.declare_dram_parameter("xt", [D, N], mmdt, isOutput=False)
    xlt = nc.declare_dram_parameter("xlt", [D, NLOC], mmdt, isOutput=False)
    w = nc.declare_dram_parameter("w", [D, D], mmdt, isOutput=False)
    b = nc.declare_dram_parameter("b", [1, 1], f32, isOutput=False)
    out = nc.declare_dram_parameter("out", [NLOC, N], f32, isOutput=True)

    with tile.TileContext(nc) as tc:
        with (
            tc.tile_pool(name="const", bufs=1) as const_pool,
            tc.tile_pool(name="stage1", bufs=1) as s1pool,
            tc.tile_pool(name="xwt", bufs=1) as xwtpool,
            tc.tile_pool(name="xtp", bufs=3) as xtpool,
            tc.tile_pool(name="outp", bufs=4) as outpool,
            tc.tile_pool(name="ps", bufs=8, space="PSUM") as pspool,
        ):
            # bias: dram scalar broadcast-DMA'd to [128, 1]
            bias = const_pool.tile([P, 1], f32)
            nc.sync.dma_start(bias, b[:].to_broadcast((P, 1)))

            # stage 1 operands, [128, KC, *] with partition = inner contraction idx
            w_r = w[:].rearrange("(ko p) m -> p ko m", p=P)
            xlt_r = xlt[:].rearrange("(ko p) m -> p ko m", p=P)
            w_sb = s1pool.tile([P, KC, D], mmdt)
            xlt_sb = s1pool.tile([P, KC, NLOC], mmdt)
            for k in range(KC):
                nc.sync.dma_start(w_sb[:, k], w_r[:, k])
                nc.sync.dma_start(xlt_sb[:, k], xlt_r[:, k])

            # stage 1: XWT[d', i] accumulated in PSUM over d chunks
            xwt_sb = xwtpool.tile([P, KC, NLOC], mmdt)
            for m in range(KC):  # d' chunk -> output partition block
                for h in range(NLOC // FD):  # i slice
                    ps = pspool.tile([P, FD], f32, tag="ps")
                    for k in range(KC):
                        nc.tensor.matmul(
                            ps,
                            lhsT=w_sb[:, k, m * P : (m + 1) * P],
                            rhs=xlt_sb[:, k, h * FD : (h + 1) * FD],
                            start=(k == 0),
                            stop=(k == KC - 1),
                        )
                    nc.any.tensor_copy(
                        out=xwt_sb[:, m, h * FD : (h + 1) * FD], in_=ps
                    )

            # stage 2: out[i, j] = XWT^T @ XT + b
            xt_r = xt[:].rearrange("(ko p) j -> p ko j", p=P)
            out_r = out[:].rearrange("(io p) j -> p io j", p=P)
            for js in range(NJ):
                xt_sb = xtpool.tile([P, KC, FD], mmdt, tag="xt")
                for k in range(KC):
                    nc.sync.dma_start(
                        xt_sb[:, k], xt_r[:, k, js * FD : (js + 1) * FD]
                    )
                for ic in range(NI):
                    ps = pspool.tile([P, FD], f32, tag="ps")
                    for k in range(KC):
                        nc.tensor.matmul(
                            ps,
                            lhsT=xwt_sb[:, k, ic * P : (ic + 1) * P],
                            rhs=xt_sb[:, k, :],
                            start=(k == 0),
                            stop=(k == KC - 1),
                        )
                    o_sb = outpool.tile([P, FD], f32, tag="o")
                    import concourse.mybir as mybir_

                    nc.scalar.activation(
                        o_sb, ps, mybir_.ActivationFunctionType.Identity, bias=bias
                    )
                    nc.sync.dma_start(out_r[:, ic, js * FD : (js + 1) * FD], o_sb)

    nc.compile()
    return nc


def _get_nc(mm_dtype_name="float32r"):
    key = mm_dtype_name
    if key not in _CACHE:
        _CACHE[key] = _build_nc(mm_dtype_name)
    return _CACHE[key]


def _round_fp32r(a):
    """Round fp32 to the nearest bf16-pair-representable value (FP32r)."""
    import ml_dtypes

    a = np.asarray(a, dtype=np.float32)
    hi = a.astype(ml_dtypes.bfloat16).astype(np.float32)
    lo = (a - hi).astype(ml_dtypes.bfloat16).astype(np.float32)
    return hi + lo


def _make_in_maps(inputs, W, b):
    X = np.ascontiguousarray(inputs, dtype=np.float32)
    XT = np.ascontiguousarray(_round_fp32r(X.T))  # (D, N)
    Wc = np.ascontiguousarray(_round_fp32r(W))
    bc = np.asarray(b, dtype=np.float32).reshape(1, 1)

    in_maps = []
    for c in range(NCORES):
        in_maps.append(
            {
                "xt": XT,
                "xlt": np.ascontiguousarray(XT[:, c * NLOC : (c + 1) * NLOC]),
                "w": Wc,
                "b": bc,
            }
        )
    return in_maps


def kernel(inputs, W, b):
    from concourse.bass_utils import run_bass_kernel_spmd

    in_maps = _make_in_maps(inputs, W, b)
    nc = _get_nc()
    res = run_bass_kernel_spmd(nc, in_maps, list(range(NCORES))).results
    return np.concatenate([res[c]["out"] for c in range(NCORES)], axis=0)


# revision 13
# speedup vs baseline: 1.0707x; 1.0707x over previous
"""Trainium2 Bass kernel for nn_Bilinear: out = (X @ W) @ X^T + b.

X: (8192, 1024) f32, W: (1024, 1024) f32, b: (1,) f32 -> out: (8192, 8192) f32.

Sharding: row-block over 8 cores. Core c computes rows [c*1024, (c+1)*1024).
Host passes X already transposed (XT, [d, N]) so the kernel needs no on-device
transposes:
  stage 1: XWT = W^T @ XlocT        (= (X_loc @ W)^T, [d', i])   via matmul(lhsT=W, rhs=XlocT)
  stage 2: out = XWT^T @ XT + b     ([i, j])                      via matmul(lhsT=XWT, rhs=XT)
Matmuls run as float32r (full-rate fp32 path on the PE for free-dim >= 256).
"""

import numpy as np

N = 8192
D = 1024
NCORES = 8
NLOC = N // NCORES  # 1024 rows per core
P = 128
KC = D // P  # 8 contraction chunks
FD = 512  # matmul free dim (one fp32 PSUM bank)
NJ = N // FD  # 16 output column slices
NI = NLOC // P  # 8 output row chunks per core

_CACHE = {}


def _build_nc(mm_dtype_name="float32r"):
    import concourse.mybir as mybir
    import concourse.tile as tile
    from concourse import bacc

    f32 = mybir.dt.float32
    mmdt = getattr(mybir.dt, mm_dtype_name)

    nc = bacc.Bacc(trn_type="TRN2")
    xt = nc.declare_dram_parameter("xt", [D, N], mmdt, isOutput=False)
    xlt = nc.declare_dram_parameter("xlt", [D, NLOC], mmdt, isOutput=False)
    w = nc.declare_dram_parameter("w", [D, D], mmdt, isOutput=False)
    b = nc.declare_dram_parameter("b", [1, 1], f32, isOutput=False)
    out = nc.declare_dram_parameter("out", [NLOC, N], f32, isOutput=True)

    with tile.TileContext(nc) as tc:
        with (
            tc.tile_pool(name="const", bufs=1) as const_pool,
            tc.tile_pool(name="stage1", bufs=1) as s1pool,
            tc.tile_pool(name="xwt", bufs=1) as xwtpool,
            tc.tile_pool(name="xtp", bufs=3) as xtpool,
            tc.tile_pool(name="outp", bufs=4) as outpool,
            tc.tile_pool(name="ps", bufs=8, space="PSUM") as pspool,
        ):
            # bias: dram scalar broadcast-DMA'd to [128, 1]
            bias = const_pool.tile([P, 1], f32)
            nc.sync.dma_start(bias, b[:].to_broadcast((P, 1)))

            # stage 1 operands, [128, KC, *] with partition = inner contraction idx
            w_r = w[:].rearrange("(ko p) m -> p ko m", p=P)
            xlt_r = xlt[:].rearrange("(ko p) m -> p ko m", p=P)
            w_sb = s1pool.tile([P, KC, D], mmdt)
            xlt_sb = s1pool.tile([P, KC, NLOC], mmdt)
            for k in range(KC):
                nc.sync.dma_start(w_sb[:, k], w_r[:, k])
                nc.sync.dma_start(xlt_sb[:, k], xlt_r[:, k])

            # stage 1: XWT[d', i] accumulated in PSUM over d chunks
            xwt_sb = xwtpool.tile([P, KC, NLOC], mmdt)
            for m in range(KC):  # d' chunk -> output partition block
                for h in range(NLOC // FD):  # i slice
                    ps = pspool.tile([P, FD], f32, tag="ps")
                    for k in range(KC):
                        nc.tensor.matmul(
                            ps,
                            lhsT=w_sb[:, k, m * P : (m + 1) * P],
                            rhs=xlt_sb[:, k, h * FD : (h + 1) * FD],
                            start=(k == 0),
                            stop=(k == KC - 1),
                        )
                    nc.any.tensor_copy(
                        out=xwt_sb[:, m, h * FD : (h + 1) * FD], in_=ps
                    )

            # stage 2: out[i, j] = XWT^T @ XT + b
            xt_r = xt[:].rearrange("(ko p) j -> p ko j", p=P)
            out_r = out[:].rearrange("(io p) j -> p io j", p=P)
            for js in range(NJ):
                xt_sb = xtpool.tile([P, KC, FD], mmdt, tag="xt")
                for k in range(KC):
                    nc.sync.dma_start(
                        xt_sb[:, k], xt_r[:, k, js * FD : (js + 1) * FD]
                    )
                for ic in range(NI):
                    ps = pspool.tile([P, FD], f32, tag="ps")
                    for k in range(KC):
                        nc.tensor.matmul(
                            ps,
                            lhsT=xwt_sb[:, k, ic * P : (ic + 1) * P],
                            rhs=xt_sb[:, k, :],
                            start=(k == 0),
                            stop=(k == KC - 1),
                        )
                    o_sb = outpool.tile([P, FD], f32, tag="o")
                    import concourse.mybir as mybir_

                    nc.scalar.activation(
                        o_sb, ps, mybir_.ActivationFunctionType.Identity, bias=bias
                    )
                    nc.sync.dma_start(out_r[:, ic, js * FD : (js + 1) * FD], o_sb)

    nc.compile()
    return nc


def _get_nc(mm_dtype_name="float32r"):
    key = mm_dtype_name
    if key not in _CACHE:
        _CACHE[key] = _build_nc(mm_dtype_name)
    return _CACHE[key]


def _round_fp32r(a):
    """Round fp32 to the nearest bf16-pair-representable value (FP32r)."""
    import ml_dtypes

    a = np.asarray(a, dtype=np.float32)
    hi = a.astype(ml_dtypes.bfloat16).astype(np.float32)
    lo = (a - hi).astype(ml_dtypes.bfloat16).astype(np.float32)
    return hi + lo


def _conv_in(a, mm_dtype_name):
    if mm_dtype_name == "float32r":
        return _round_fp32r(a)
    if mm_dtype_name == "bfloat16":
        import ml_dtypes

        return np.asarray(a, dtype=np.float32).astype(ml_dtypes.bfloat16)
    return np.asarray(a, dtype=np.float32)


def _make_in_maps(inputs, W, b, mm_dtype_name="float32r"):
    X = np.ascontiguousarray(inputs, dtype=np.float32)
    XT = np.ascontiguousarray(_conv_in(X.T, mm_dtype_name))  # (D, N)
    Wc = np.ascontiguousarray(_conv_in(W, mm_dtype_name))
    bc = np.asarray(b, dtype=np.float32).reshape(1, 1)

    in_maps = []
    for c in range(NCORES):
        in_maps.append(
            {
                "xt": XT,
                "xlt": np.ascontiguousarray(XT[:, c * NLOC : (c + 1) * NLOC]),
                "w": Wc,
                "b": bc,
            }
        )
    return in_maps


def kernel(inputs, W, b):
    from concourse.bass_utils import run_bass_kernel_spmd

    in_maps = _make_in_maps(inputs, W, b)
    nc = _get_nc()
    res = run_bass_kernel_spmd(nc, in_maps, list(range(NCORES))).results
    return np.concatenate([res[c]["out"] for c in range(NCORES)], axis=0)
